# revision 7
# baseline (speedup 1.0000x reference)
"""Multi-head attention (B=2, S=2048, D=1024, H=16) on 8 trn2 NeuronCores.

Sharding: data-parallel over batch (2 groups of 4 cores), tensor-parallel over
heads within a group (4 heads/core).  Each core computes a partial output
(its heads' contribution through its W_o column shard); the host sums the 4
partials per batch element.

Per-core kernel layout choices:
  - q, k are produced TRANSPOSED ([d_local, s], head dim on partitions) so the
    scores matmul s^T[s_k, s_q] needs no transpose: lhsT = kT tile, rhs = qT.
    Dk=64 < 128, so the two heads of a pair are row-packed into the PE array
    via tile_position=(0,0)/(64,0) and run concurrently.
  - v is produced in natural layout [s, d] with a fused ones-column so the AV
    matmul (lhsT = v_aug stationary [128, 65], rhs = exp tile) emits both the
    attention output (rows 0:64, transposed) and the softmax denominator
    (row 64) in one accumulation chain.  Softmax skips max-subtraction
    (scores ~ N(0,1) after the 1/8 scale; exp cannot overflow in fp32).
  - exp runs on ACT directly from PSUM in one [128, 2048]-wide op per k-tile
    (both heads), with the 1/sqrt(64) scale folded into the activation.
"""

import os
from contextlib import ExitStack

import numpy as np

B = 2
S = 2048
DM = 1024
H = 16
DK = 64
P = 128
HC = 4            # heads per core
DO = HC * DK      # 256: local output dim of q/k/v projections
DI_T = DM // P    # 8 contraction tiles for projections
S_T = S // P      # 16
QC = 1024         # s_q chunk processed per attention block
N_QC = S // QC

_PROGRAM = None


def _build_program():
    import concourse.mybir as mybir
    import concourse.tile as tile
    from concourse import bacc

    f32 = mybir.dt.float32
    nc = bacc.Bacc("TRN2", target_bir_lowering=False, debug=False)

    qt_d = nc.dram_tensor("QT", [DM, S], f32, kind="ExternalInput").ap()
    kt_d = nc.dram_tensor("KT", [DM, S], f32, kind="ExternalInput").ap()
    vt_d = nc.dram_tensor("VT", [DM, S], f32, kind="ExternalInput").ap()
    wqt_d = nc.dram_tensor("WQT", [DM, DO], f32, kind="ExternalInput").ap()
    wkt_d = nc.dram_tensor("WKT", [DM, DO], f32, kind="ExternalInput").ap()
    wvt_d = nc.dram_tensor("WVT", [DM, DO], f32, kind="ExternalInput").ap()
    wot_d = nc.dram_tensor("WOT", [DO, DM], f32, kind="ExternalInput").ap()
    out_d = nc.dram_tensor("OUT", [S, DM], f32, kind="ExternalOutput").ap()

    with tile.TileContext(nc) as tc, ExitStack() as ctx:
        _emit(ctx, tc, qt_d, kt_d, vt_d, wqt_d, wkt_d, wvt_d, wot_d, out_d)
    nc.compile()
    return nc


def _emit(ctx, tc, qt_d, kt_d, vt_d, wqt_d, wkt_d, wvt_d, wot_d, out_d):
    import concourse.mybir as mybir

    nc = tc.nc
    f32 = mybir.dt.float32
    Exp = mybir.ActivationFunctionType.Exp

    consts = ctx.enter_context(tc.tile_pool(name="consts", bufs=1))
    staging = ctx.enter_context(tc.tile_pool(name="staging", bufs=3))
    exp_pool = ctx.enter_context(tc.tile_pool(name="exp", bufs=3))
    smalls = ctx.enter_context(tc.tile_pool(name="smalls", bufs=2))
    ostage = ctx.enter_context(tc.tile_pool(name="ostage", bufs=2))

    # persistent SBUF tensors
    wq_sb = consts.tile([P, DI_T, DO], f32, tag="wq")
    wk_sb = consts.tile([P, DI_T, DO], f32, tag="wk")
    wv_sb = consts.tile([P, DI_T, DO], f32, tag="wv")
    wo_sb = consts.tile([P, 2, DM], f32, tag="wo")
    qT_sb = consts.tile([P, 2, S], f32, tag="qT")     # [p, ot, s]; o_local = ot*128+p
    kT_sb = consts.tile([P, 2, S], f32, tag="kT")
    vab_sb = consts.tile([P, S_T, HC, DK + 1], f32, tag="vab")
    attnT_sb = consts.tile([P, 2, S], f32, tag="attnT")
    stage_sb = consts.tile([64, 2, S], f32, tag="oddstage")
    ones_sb = consts.tile([1, 64], f32, tag="ones")

    nc.sync.dma_start(wq_sb[:], wqt_d.rearrange("(t p) o -> p t o", p=P))
    nc.sync.dma_start(wk_sb[:], wkt_d.rearrange("(t p) o -> p t o", p=P))
    nc.sync.dma_start(wv_sb[:], wvt_d.rearrange("(t p) o -> p t o", p=P))
    nc.sync.dma_start(wo_sb[:], wot_d.rearrange("(t p) o -> p t o", p=P))
    nc.vector.memset(vab_sb[:, :, :, DK : DK + 1], 1.0)
    nc.vector.memset(ones_sb[:], 1.0)

    # ---------------- projections ----------------
    with tc.tile_pool(name="psum_proj", bufs=8, space="PSUM") as psum_proj:
        # q, k: transposed outputs qT[o_local, s] = sum_din W[o,din] * X[s,din]
        for src, wsb, dst in ((qt_d, wq_sb, qT_sb), (kt_d, wk_sb, kT_sb)):
            ps = [psum_proj.tile([P, 512], f32, tag="proj", name=f"pj{i}") for i in range(8)]
            for t in range(DI_T):
                stg = staging.tile([P, S], f32, tag="instage")
                nc.sync.dma_start(stg[:], src[t * P : (t + 1) * P, :])
                for ot in range(2):
                    for sc in range(4):
                        nc.tensor.matmul(
                            ps[ot * 4 + sc][:],
                            wsb[:, t, ot * P : (ot + 1) * P],
                            stg[:, sc * 512 : (sc + 1) * 512],
                            start=(t == 0),
                            stop=(t == DI_T - 1),
                        )
            for ot in range(2):
                for sc in range(4):
                    nc.vector.tensor_copy(
                        dst[:, ot, sc * 512 : (sc + 1) * 512], ps[ot * 4 + sc][:]
                    )

        # v: natural layout v[s, o], written into vab (ones column preset).
        # One accumulation group per PSUM bank, so process s in two halves
        # (8 banks each).
        for half in range(2):
            s0 = half * (S // 2)
            ps = [
                psum_proj.tile([P, 256], f32, tag="proj", name=f"pv{half}_{i}")
                for i in range(8)
            ]
            for t in range(DI_T):
                stg = staging.tile([P, S // 2], f32, tag="instage")
                nc.sync.dma_start(stg[:], vt_d[t * P : (t + 1) * P, s0 : s0 + S // 2])
                for si in range(8):
                    nc.tensor.matmul(
                        ps[si][:],
                        stg[:, si * P : (si + 1) * P],
                        wv_sb[:, t, :],
                        start=(t == 0),
                        stop=(t == DI_T - 1),
                    )
            for si in range(8):
                nc.vector.tensor_copy(
                    vab_sb[:, half * 8 + si, :, 0:DK],
                    ps[si][:].rearrange("p (h d) -> p h d", d=DK),
                )

    # ---------------- attention ----------------
    with (
        tc.tile_pool(name="psum_s", bufs=1, space="PSUM") as psum_s_pool,
        tc.tile_pool(name="psum_av", bufs=2, space="PSUM") as psum_av_pool,
    ):
        for qc in range(N_QC):
            q0 = qc * QC
            for hp in range(2):
                av = [psum_av_pool.tile([P, QC], f32, tag="av", name=f"av{j}") for j in range(2)]
                for t in range(S_T):
                    ps_s = psum_s_pool.tile([P, 2, QC], f32, tag="scores")
                    for j in range(2):
                        hb = j * 64
                        for qn in range(QC // 512):
                            nc.tensor.matmul(
                                ps_s[:, j, qn * 512 : (qn + 1) * 512],
                                kT_sb[hb : hb + 64, hp, t * P : (t + 1) * P],
                                qT_sb[hb : hb + 64, hp, q0 + qn * 512 : q0 + (qn + 1) * 512],
                                start=True,
                                stop=True,
                                tile_position=(hb, 0),
                            )
                    ex = exp_pool.tile([P, 2, QC], f32, tag="exp")
                    nc.scalar.activation(ex[:], ps_s[:], Exp, scale=0.125)
                    for j in range(2):
                        for qn in range(QC // 512):
                            nc.tensor.matmul(
                                av[j][0 : DK + 1, qn * 512 : (qn + 1) * 512],
                                vab_sb[:, t, 2 * hp + j, :],
                                ex[:, j, qn * 512 : (qn + 1) * 512],
                                start=(t == 0),
                                stop=(t == S_T - 1),
                            )
                # epilogue: divide by denominators (row DK of av).
                # Broadcast the denom row across 64 partitions with a K=1
                # rank-1 matmul (ones^T (x) den_row) into a PSUM tile that
                # shares the scores pool slot (free between k-tile loops).
                for j in range(2):
                    lh = 2 * hp + j
                    den_row = smalls.tile([1, QC], f32, tag="den")
                    nc.vector.tensor_copy(den_row[:], av[j][DK : DK + 1, :])
                    den_b = psum_s_pool.tile([64, QC], f32, tag="scores", name="den_b")
                    for qn in range(QC // 512):
                        nc.tensor.matmul(
                            den_b[:, qn * 512 : (qn + 1) * 512],
                            ones_sb[:],
                            den_row[0:1, qn * 512 : (qn + 1) * 512],
                            start=True,
                            stop=True,
                        )
                    rec_b = smalls.tile([64, QC], f32, tag="recb")
                    nc.vector.reciprocal_approx_fast(rec_b[:], den_b[:])
                    if lh % 2 == 0:
                        nc.vector.tensor_mul(
                            attnT_sb[0:64, lh // 2, q0 : q0 + QC],
                            av[j][0:DK, :],
                            rec_b[:],
                        )
                    else:
                        nc.vector.tensor_mul(
                            stage_sb[:, lh // 2, q0 : q0 + QC],
                            av[j][0:DK, :],
                            rec_b[:],
                        )
                        nc.sync.dma_start(
                            attnT_sb[64:128, lh // 2, q0 : q0 + QC],
                            stage_sb[:, lh // 2, q0 : q0 + QC],
                        )

        # ---------------- output projection ----------------
        for st in range(S_T):
            po = psum_av_pool.tile([P, QC], f32, tag="av")
            for ot in range(2):
                for col in range(2):
                    nc.tensor.matmul(
                        po[:, col * 512 : (col + 1) * 512],
                        attnT_sb[:, ot, st * P : (st + 1) * P],
                        wo_sb[:, ot, col * 512 : (col + 1) * 512],
                        start=(ot == 0),
                        stop=(ot == 1),
                    )
            ob = ostage.tile([P, DM], f32, tag="ostage")
            nc.vector.tensor_copy(ob[:], po[:])
            nc.sync.dma_start(out_d[st * P : (st + 1) * P, :], ob[:])


def _get_program():
    global _PROGRAM
    if _PROGRAM is None:
        _PROGRAM = _build_program()
    return _PROGRAM


def make_in_maps(Q, K, V, W_q, W_k, W_v, W_o):
    """Per-core input dicts: core c -> batch c//4, heads (c%4)*4 ... +4."""
    in_maps = []
    for c in range(8):
        b, g = c // 4, c % 4
        sl = slice(g * DO, (g + 1) * DO)
        in_maps.append(
            {
                "QT": np.ascontiguousarray(Q[b].T, dtype=np.float32),
                "KT": np.ascontiguousarray(K[b].T, dtype=np.float32),
                "VT": np.ascontiguousarray(V[b].T, dtype=np.float32),
                "WQT": np.ascontiguousarray(W_q[sl, :].T, dtype=np.float32),
                "WKT": np.ascontiguousarray(W_k[sl, :].T, dtype=np.float32),
                "WVT": np.ascontiguousarray(W_v[sl, :].T, dtype=np.float32),
                "WOT": np.ascontiguousarray(W_o[:, sl].T, dtype=np.float32),
            }
        )
    return in_maps


def combine_outputs(outs):
    """outs: list of 8 [S, DM] partials -> [B, S, DM]."""
    return np.stack(
        [
            outs[0] + outs[1] + outs[2] + outs[3],
            outs[4] + outs[5] + outs[6] + outs[7],
        ]
    ).astype(np.float32)


def kernel(Q, K, V, W_q, W_k, W_v, W_o):
    from concourse.bass_utils import run_bass_kernel_spmd

    Q = np.asarray(Q)
    K = np.asarray(K)
    V = np.asarray(V)
    nc = _get_program()
    in_maps = make_in_maps(Q, K, V, np.asarray(W_q), np.asarray(W_k), np.asarray(W_v), np.asarray(W_o))
    res = run_bass_kernel_spmd(nc, in_maps, core_ids=list(range(8)))
    return combine_outputs([res.results[c]["OUT"] for c in range(8)])


# revision 10
# speedup vs baseline: 1.7233x; 1.7233x over previous
"""Multi-head attention (B=2, S=2048, D=1024, H=16) on 8 trn2 NeuronCores.

Sharding: data-parallel over batch (2 groups of 4 cores), tensor-parallel over
heads within a group (4 heads/core).  Each core computes a partial output
(its heads' contribution through its W_o column shard); the host sums the 4
partials per batch element.

Per-core kernel layout choices:
  - q, k are produced TRANSPOSED ([d_local, s], head dim on partitions) so the
    scores matmul s^T[s_k, s_q] needs no transpose: lhsT = kT tile, rhs = qT.
    Dk=64 < 128, so the two heads of a pair are row-packed into the PE array
    via tile_position=(0,0)/(64,0) and run concurrently.
  - v is produced in natural layout [s, d] with a fused ones-column so the AV
    matmul (lhsT = v_aug stationary [128, 65], rhs = exp tile) emits both the
    attention output (rows 0:64, transposed) and the softmax denominator
    (row 64) in one accumulation chain.  Softmax skips max-subtraction
    (scores ~ N(0,1) after the 1/8 scale; exp cannot overflow in fp32).
  - exp runs on ACT directly from PSUM in one [128, 2048]-wide op per k-tile
    (both heads), with the 1/sqrt(64) scale folded into the activation.
"""

import os
from contextlib import ExitStack

import numpy as np

B = 2
S = 2048
DM = 1024
H = 16
DK = 64
P = 128
HC = 4            # heads per core
DO = HC * DK      # 256: local output dim of q/k/v projections
DI_T = DM // P    # 8 contraction tiles for projections
S_T = S // P      # 16
QC = 1024         # s_q chunk processed per attention block
N_QC = S // QC

MM_BF16 = True    # matmul inputs in fp16 (fp32 PSUM accumulation everywhere)

_PROGRAM = None


def _build_program():
    import concourse.mybir as mybir
    import concourse.tile as tile
    from concourse import bacc

    f32 = mybir.dt.float32
    mmdt = mybir.dt.float16 if MM_BF16 else f32
    nc = bacc.Bacc("TRN2", target_bir_lowering=False, debug=False)

    qt_d = nc.dram_tensor("QT", [DM, S], mmdt, kind="ExternalInput").ap()
    kt_d = nc.dram_tensor("KT", [DM, S], mmdt, kind="ExternalInput").ap()
    vt_d = nc.dram_tensor("VT", [DM, S], mmdt, kind="ExternalInput").ap()
    wqt_d = nc.dram_tensor("WQT", [DM, DO], mmdt, kind="ExternalInput").ap()
    wkt_d = nc.dram_tensor("WKT", [DM, DO], mmdt, kind="ExternalInput").ap()
    wvt_d = nc.dram_tensor("WVT", [DM, DO], mmdt, kind="ExternalInput").ap()
    wot_d = nc.dram_tensor("WOT", [DO, DM], mmdt, kind="ExternalInput").ap()
    out_d = nc.dram_tensor("OUT", [S, DM], f32, kind="ExternalOutput").ap()

    with tile.TileContext(nc) as tc, ExitStack() as ctx:
        _emit(ctx, tc, qt_d, kt_d, vt_d, wqt_d, wkt_d, wvt_d, wot_d, out_d)
    nc.compile()
    return nc


def _emit(ctx, tc, qt_d, kt_d, vt_d, wqt_d, wkt_d, wvt_d, wot_d, out_d):
    import concourse.mybir as mybir

    nc = tc.nc
    f32 = mybir.dt.float32
    mmdt = mybir.dt.float16 if MM_BF16 else f32
    Exp = mybir.ActivationFunctionType.Exp

    consts = ctx.enter_context(tc.tile_pool(name="consts", bufs=1))
    staging = ctx.enter_context(tc.tile_pool(name="staging", bufs=3))
    exp_pool = ctx.enter_context(tc.tile_pool(name="exp", bufs=3))
    smalls = ctx.enter_context(tc.tile_pool(name="smalls", bufs=2))
    ostage = ctx.enter_context(tc.tile_pool(name="ostage", bufs=2))

    # persistent SBUF tensors
    wq_sb = consts.tile([P, DI_T, DO], mmdt, tag="wq")
    wk_sb = consts.tile([P, DI_T, DO], mmdt, tag="wk")
    wv_sb = consts.tile([P, DI_T, DO], mmdt, tag="wv")
    wo_sb = consts.tile([P, 2, DM], mmdt, tag="wo")
    qT_sb = consts.tile([P, 2, S], mmdt, tag="qT")    # [p, ot, s]; o_local = ot*128+p
    kT_sb = consts.tile([P, 2, S], mmdt, tag="kT")
    vab_sb = consts.tile([P, S_T, HC, DK + 1], mmdt, tag="vab")
    attnT_sb = consts.tile([P, 2, S], mmdt, tag="attnT")
    stage_sb = consts.tile([64, 2, S], mmdt, tag="oddstage")
    ones_sb = consts.tile([1, 64], f32, tag="ones")

    nc.sync.dma_start(wq_sb[:], wqt_d.rearrange("(t p) o -> p t o", p=P))
    nc.sync.dma_start(wk_sb[:], wkt_d.rearrange("(t p) o -> p t o", p=P))
    nc.sync.dma_start(wv_sb[:], wvt_d.rearrange("(t p) o -> p t o", p=P))
    nc.sync.dma_start(wo_sb[:], wot_d.rearrange("(t p) o -> p t o", p=P))
    nc.vector.memset(vab_sb[:, :, :, DK : DK + 1], 1.0)
    nc.vector.memset(ones_sb[:], 1.0)

    # ---------------- projections ----------------
    with tc.tile_pool(name="psum_proj", bufs=8, space="PSUM") as psum_proj:
        # q, k: transposed outputs qT[o_local, s] = sum_din W[o,din] * X[s,din]
        for src, wsb, dst in ((qt_d, wq_sb, qT_sb), (kt_d, wk_sb, kT_sb)):
            ps = [psum_proj.tile([P, 512], f32, tag="proj", name=f"pj{i}") for i in range(8)]
            for t in range(DI_T):
                stg = staging.tile([P, S], mmdt, tag="instage")
                nc.sync.dma_start(stg[:], src[t * P : (t + 1) * P, :])
                for ot in range(2):
                    for sc in range(4):
                        nc.tensor.matmul(
                            ps[ot * 4 + sc][:],
                            wsb[:, t, ot * P : (ot + 1) * P],
                            stg[:, sc * 512 : (sc + 1) * 512],
                            start=(t == 0),
                            stop=(t == DI_T - 1),
                        )
            for ot in range(2):
                for sc in range(4):
                    nc.vector.tensor_copy(
                        dst[:, ot, sc * 512 : (sc + 1) * 512], ps[ot * 4 + sc][:]
                    )

        # v: natural layout v[s, o], written into vab (ones column preset).
        # One accumulation group per PSUM bank, so process s in two halves
        # (8 banks each).
        for half in range(2):
            s0 = half * (S // 2)
            ps = [
                psum_proj.tile([P, 256], f32, tag="proj", name=f"pv{half}_{i}")
                for i in range(8)
            ]
            for t in range(DI_T):
                stg = staging.tile([P, S // 2], mmdt, tag="instage")
                nc.sync.dma_start(stg[:], vt_d[t * P : (t + 1) * P, s0 : s0 + S // 2])
                for si in range(8):
                    nc.tensor.matmul(
                        ps[si][:],
                        stg[:, si * P : (si + 1) * P],
                        wv_sb[:, t, :],
                        start=(t == 0),
                        stop=(t == DI_T - 1),
                    )
            for si in range(8):
                nc.vector.tensor_copy(
                    vab_sb[:, half * 8 + si, :, 0:DK],
                    ps[si][:].rearrange("p (h d) -> p h d", d=DK),
                )

    # ---------------- attention ----------------
    with (
        tc.tile_pool(name="psum_s", bufs=1, space="PSUM") as psum_s_pool,
        tc.tile_pool(name="psum_av", bufs=2, space="PSUM") as psum_av_pool,
    ):
        for qc in range(N_QC):
            q0 = qc * QC
            for hp in range(2):
                av = [psum_av_pool.tile([P, QC], f32, tag="av", name=f"av{j}") for j in range(2)]
                for t in range(S_T):
                    ps_s = psum_s_pool.tile([P, 2, QC], f32, tag="scores")
                    for j in range(2):
                        hb = j * 64
                        for qn in range(QC // 512):
                            nc.tensor.matmul(
                                ps_s[:, j, qn * 512 : (qn + 1) * 512],
                                kT_sb[hb : hb + 64, hp, t * P : (t + 1) * P],
                                qT_sb[hb : hb + 64, hp, q0 + qn * 512 : q0 + (qn + 1) * 512],
                                start=True,
                                stop=True,
                                tile_position=(hb, 0),
                            )
                    ex = exp_pool.tile([P, 2, QC], mmdt, tag="exp")
                    nc.scalar.activation(ex[:], ps_s[:], Exp, scale=0.125)
                    for j in range(2):
                        for qn in range(QC // 512):
                            nc.tensor.matmul(
                                av[j][0 : DK + 1, qn * 512 : (qn + 1) * 512],
                                vab_sb[:, t, 2 * hp + j, :],
                                ex[:, j, qn * 512 : (qn + 1) * 512],
                                start=(t == 0),
                                stop=(t == S_T - 1),
                            )
                # epilogue: divide by denominators (row DK of av).
                # Broadcast the denom row across 64 partitions with a K=1
                # rank-1 matmul (ones^T (x) den_row) into a PSUM tile that
                # shares the scores pool slot (free between k-tile loops).
                for j in range(2):
                    lh = 2 * hp + j
                    den_row = smalls.tile([1, QC], f32, tag="den")
                    nc.vector.tensor_copy(den_row[:], av[j][DK : DK + 1, :])
                    den_b = psum_s_pool.tile([64, QC], f32, tag="scores", name="den_b")
                    for qn in range(QC // 512):
                        nc.tensor.matmul(
                            den_b[:, qn * 512 : (qn + 1) * 512],
                            ones_sb[:],
                            den_row[0:1, qn * 512 : (qn + 1) * 512],
                            start=True,
                            stop=True,
                        )
                    rec_b = smalls.tile([64, QC], f32, tag="recb")
                    nc.vector.reciprocal_approx_fast(rec_b[:], den_b[:])
                    if lh % 2 == 0:
                        nc.vector.tensor_mul(
                            attnT_sb[0:64, lh // 2, q0 : q0 + QC],
                            av[j][0:DK, :],
                            rec_b[:],
                        )
                    else:
                        nc.vector.tensor_mul(
                            stage_sb[:, lh // 2, q0 : q0 + QC],
                            av[j][0:DK, :],
                            rec_b[:],
                        )
                        nc.sync.dma_start(
                            attnT_sb[64:128, lh // 2, q0 : q0 + QC],
                            stage_sb[:, lh // 2, q0 : q0 + QC],
                        )

        # ---------------- output projection ----------------
        for st in range(S_T):
            po = psum_av_pool.tile([P, QC], f32, tag="av")
            for ot in range(2):
                for col in range(2):
                    nc.tensor.matmul(
                        po[:, col * 512 : (col + 1) * 512],
                        attnT_sb[:, ot, st * P : (st + 1) * P],
                        wo_sb[:, ot, col * 512 : (col + 1) * 512],
                        start=(ot == 0),
                        stop=(ot == 1),
                    )
            ob = ostage.tile([P, DM], f32, tag="ostage")
            nc.vector.tensor_copy(ob[:], po[:])
            nc.sync.dma_start(out_d[st * P : (st + 1) * P, :], ob[:])


def _get_program():
    global _PROGRAM
    if _PROGRAM is None:
        _PROGRAM = _build_program()
    return _PROGRAM


def make_in_maps(Q, K, V, W_q, W_k, W_v, W_o):
    """Per-core input dicts: core c -> batch c//4, heads (c%4)*4 ... +4."""
    mmdt = np.float16 if MM_BF16 else np.float32
    in_maps = []
    for c in range(8):
        b, g = c // 4, c % 4
        sl = slice(g * DO, (g + 1) * DO)
        in_maps.append(
            {
                "QT": np.ascontiguousarray(Q[b].T).astype(mmdt),
                "KT": np.ascontiguousarray(K[b].T).astype(mmdt),
                "VT": np.ascontiguousarray(V[b].T).astype(mmdt),
                "WQT": np.ascontiguousarray(W_q[sl, :].T).astype(mmdt),
                "WKT": np.ascontiguousarray(W_k[sl, :].T).astype(mmdt),
                "WVT": np.ascontiguousarray(W_v[sl, :].T).astype(mmdt),
                "WOT": np.ascontiguousarray(W_o[:, sl].T).astype(mmdt),
            }
        )
    return in_maps


def combine_outputs(outs):
    """outs: list of 8 [S, DM] partials -> [B, S, DM]."""
    return np.stack(
        [
            outs[0] + outs[1] + outs[2] + outs[3],
            outs[4] + outs[5] + outs[6] + outs[7],
        ]
    ).astype(np.float32)


def kernel(Q, K, V, W_q, W_k, W_v, W_o):
    from concourse.bass_utils import run_bass_kernel_spmd

    Q = np.asarray(Q)
    K = np.asarray(K)
    V = np.asarray(V)
    nc = _get_program()
    in_maps = make_in_maps(Q, K, V, np.asarray(W_q), np.asarray(W_k), np.asarray(W_v), np.asarray(W_o))
    res = run_bass_kernel_spmd(nc, in_maps, core_ids=list(range(8)))
    return combine_outputs([res.results[c]["OUT"] for c in range(8)])
